# revision 1
# baseline (speedup 1.0000x reference)
"""Trainium2 Bass kernel for nn_CrossAttention (b=2, s1=2048, s2=3072, 16 heads, d=64).

Sharding: 8 cores = 2 batches x 4 head-groups (4 heads each). Each core:
  - computes q = LN(x @ WqT + bq)*scale, k = LN(y @ WkT + bk), v = y @ WvT + bv
    for its 4 heads from the full x[b] and the *valid-key-compacted* y[b],
  - computes transposed attention scores (keys on partitions) so the padding
    mask is a per-partition additive bias fused into the ACT exp eviction,
  - accumulates ctxT via PE matmuls with v as the stationary operand; a ones
    column appended to v yields softmax denominators for free,
  - computes the partial output projection for its head group.
Host sums the 4 partials per batch and adds bo.

Attention blocks are software-pipelined (scores of block n+1 are emitted
before the pv/normalize of block n) so the PE never sits behind the
ACT-bound exp evictions long enough for HAM to re-throttle the clock.
"""

import math
import os

import ml_dtypes
import numpy as np

import concourse.bacc as bacc
import concourse.bass as bass
import concourse.tile as tile
from concourse import mybir
from concourse.bass_utils import run_bass_kernel_spmd
from concourse.masks import make_identity

F32 = mybir.dt.float32
F32R = mybir.dt.float32r
BF16 = mybir.dt.bfloat16

P = 128
D = 64
EPS = 1e-6
MASK_NEG = -1e9

# Matmul input dtype: bf16 = 1 cycle/row + FWL; f32r lowers to fp32-HIGH at
# ~2 cycles/row; f32 = 4 cycles/row.
MM_DT = {"bf16": BF16, "f32r": F32R, "f32": F32}[os.environ.get("K_MM_DT", "bf16")]
# Probability / v dtype for the pv matmul.
PV_DT = BF16 if os.environ.get("K_PV_DT", "bf16") == "bf16" else F32

LAST_EXEC_NS = None


def _bcast_row(ap, nparts):
    """AP reading a (1, N) slice broadcast to (nparts, N) via a 0-stride
    partition dim (same trick as tile_groupnorm's bias load)."""
    return bass.AP(
        tensor=ap.tensor, offset=ap.offset, ap=[[0, nparts]] + list(ap.ap[1:])
    )


def _build_nc(S1, S2P, C, flags):
    G = 4 * D  # 256 channels per core (4 heads)
    NI = S1 // P
    NJ = S2P // P
    CT = C // P
    IBW = 1024 if NJ <= 16 else 512  # i-block width for the attention phase
    NIB = S1 // IBW
    NC2 = IBW // 512
    AF = mybir.ActivationFunctionType
    OP = mybir.AluOpType

    TR_DT = BF16 if MM_DT == BF16 else F32  # LN-output / transpose dtype

    nc = bacc.Bacc("TRN2", target_bir_lowering=False, debug=False)

    xT_d = nc.dram_tensor("xT", [C, S1], MM_DT, kind="ExternalInput")
    yT_d = nc.dram_tensor("yT", [C, S2P], MM_DT, kind="ExternalInput")
    wqT_d = nc.dram_tensor("wqT", [C, G], MM_DT, kind="ExternalInput")
    wkT_d = nc.dram_tensor("wkT", [C, G], MM_DT, kind="ExternalInput")
    wvT_d = nc.dram_tensor("wvT", [C, G], MM_DT, kind="ExternalInput")
    woT_d = nc.dram_tensor("woT", [G, C], MM_DT, kind="ExternalInput")
    vec_d = nc.dram_tensor("vec", [8, G], F32, kind="ExternalInput")
    mask_d = nc.dram_tensor("maskb", [S2P], F32, kind="ExternalInput")
    out_d = nc.dram_tensor("out", [S1, C], F32, kind="ExternalOutput")

    VROW = {"bq": 0, "bk": 1, "bv": 2, "qw": 3, "qb": 4, "kw": 5, "kb": 6}

    with tile.TileContext(nc) as tc:
        with (
            tc.tile_pool(name="singles", bufs=1) as singles,
            tc.tile_pool(name="persist", bufs=1) as persist,
        ):
            ident = singles.tile([P, P], TR_DT, tag="ident")
            eps_sb = singles.tile([P, 1], F32, tag="eps")
            nc.vector.memset(eps_sb, EPS)
            mask_sb = singles.tile([P, NJ], F32, tag="mask")
            vec_sb = {}
            for nm in [k for k, use in flags.items() if use]:
                vec_sb[nm] = singles.tile(
                    [P, G], F32, tag=f"vec_{nm}", name=f"vec_{nm}"
                )

            qT = [
                persist.tile([P, S1], MM_DT, tag=f"qT{i}", name=f"qT{i}")
                for i in range(2)
            ]
            kT = [
                persist.tile([P, S2P], MM_DT, tag=f"kT{i}", name=f"kT{i}")
                for i in range(2)
            ]
            v_sb = persist.tile([P, NJ, 4 * (D + 1)], PV_DT, tag="v")
            ctxT = [
                persist.tile([P, S1], MM_DT, tag=f"ctxT{i}", name=f"ctxT{i}")
                for i in range(2)
            ]
            # ones column per head for the softmax denominator
            v4 = v_sb.rearrange("p j (h e) -> p j h e", e=D + 1)
            nc.vector.memset(v4[:, :, :, D : D + 1], 1.0)

            def ln_project(act_sb, w_sb, raw, smv, ntiles, scale_fold, bias_nm):
                """act_sb: (P, CT, S) transposed activations; produces raw
                (P, ntiles, G) = act.T @ W + bias, plus per-head sum/sumsq in
                smv (P, ntiles, 4, 2); returns batched (rs, -mu*rs)."""
                for it in range(ntiles):
                    ps = psA.tile([P, G], F32, tag="psA")
                    for ct in range(CT):
                        nc.tensor.matmul(
                            ps,
                            lhsT=act_sb[:, ct, it * P : (it + 1) * P],
                            rhs=w_sb[:, ct, :],
                            start=(ct == 0),
                            stop=(ct == CT - 1),
                        )
                    dst = raw[:, it, :]
                    if bias_nm in vec_sb:
                        nc.vector.tensor_add(out=dst, in0=ps, in1=vec_sb[bias_nm])
                    else:
                        nc.scalar.copy(out=dst, in_=ps)
                    dst3 = dst.rearrange("p (h e) -> p h e", e=D)
                    nc.vector.tensor_reduce(
                        out=smv[:, it, :, 0:1],
                        in_=dst3,
                        axis=mybir.AxisListType.X,
                        op=OP.add,
                    )
                    sq = work.tile([P, G], F32, tag="sq")
                    nc.vector.tensor_mul(out=sq, in0=dst, in1=dst)
                    nc.vector.tensor_reduce(
                        out=smv[:, it, :, 1:2],
                        in_=sq.rearrange("p (h e) -> p h e", e=D),
                        axis=mybir.AxisListType.X,
                        op=OP.add,
                    )
                # batched: mu = s/D; var = sq/D - mu^2; rs = scale/sqrt(var+eps)
                n4 = ntiles * 4
                smv_flat = smv.rearrange("p i h s -> p (i h s)")
                mu = work.tile([P, n4], F32, tag=f"mu{bias_nm}")
                nc.vector.tensor_scalar_mul(
                    out=mu, in0=smv_flat[:, 0::2], scalar1=1.0 / D
                )
                var = work.tile([P, n4], F32, tag=f"va{bias_nm}")
                nc.vector.tensor_scalar_mul(
                    out=var, in0=smv_flat[:, 1::2], scalar1=1.0 / D
                )
                mu2 = work.tile([P, n4], F32, tag=f"m2{bias_nm}")
                nc.vector.tensor_mul(out=mu2, in0=mu, in1=mu)
                nc.vector.tensor_sub(out=var, in0=var, in1=mu2)
                sd = work.tile([P, n4], F32, tag=f"sd{bias_nm}")
                nc.scalar.activation(
                    out=sd, in_=var, func=AF.Sqrt, bias=eps_sb, scale=1.0
                )
                rs = work.tile([P, n4], F32, tag=f"rs{bias_nm}")
                nc.vector.reciprocal(out=rs, in_=sd)
                if scale_fold != 1.0:
                    nc.vector.tensor_scalar_mul(out=rs, in0=rs, scalar1=scale_fold)
                nm_ = work.tile([P, n4], F32, tag=f"nm{bias_nm}")
                nc.vector.tensor_mul(out=nm_, in0=mu, in1=rs)
                nc.vector.tensor_scalar_mul(out=nm_, in0=nm_, scalar1=-1.0)
                return rs, nm_

            def ln_apply_transpose(raw, rs, nm_, ntiles, w_nm, b_nm, dstT):
                for it in range(ntiles):
                    qa = work.tile([P, G], TR_DT, tag="qa")
                    for h4 in range(4):
                        i4 = it * 4 + h4
                        eng = nc.gpsimd if it % 2 == 0 else nc.vector
                        eng.tensor_scalar(
                            out=qa[:, h4 * D : (h4 + 1) * D],
                            in0=raw[:, it, h4 * D : (h4 + 1) * D],
                            scalar1=rs[:, i4 : i4 + 1],
                            scalar2=nm_[:, i4 : i4 + 1],
                            op0=OP.mult,
                            op1=OP.add,
                        )
                    if w_nm in vec_sb:
                        nc.vector.tensor_mul(out=qa, in0=qa, in1=vec_sb[w_nm])
                    if b_nm in vec_sb:
                        nc.vector.tensor_add(out=qa, in0=qa, in1=vec_sb[b_nm])
                    for half in range(2):
                        pt = psT.tile([P, P], TR_DT, tag="ptr")
                        nc.tensor.transpose(pt, qa[:, half * P : (half + 1) * P], ident)
                        if it % 2 == 0:
                            nc.scalar.copy(
                                out=dstT[half][:, it * P : (it + 1) * P], in_=pt
                            )
                        else:
                            nc.vector.tensor_copy(
                                out=dstT[half][:, it * P : (it + 1) * P], in_=pt
                            )

            # ------- Phase 1+2: q/k/v projections + LN + transposes ----------
            # One scope so the Tile scheduler interleaves q-, k- and v-side
            # matmuls with the DVE LayerNorm work (PE stays dense/HAM warm).
            # Input DMAs spread across engine queues: each engine owns one
            # dynamic HW queue, and a single queue serializes the loads.
            with (
                tc.tile_pool(name="ph1", bufs=1) as ph1,
                tc.tile_pool(name="work", bufs=3) as work,
                tc.tile_pool(name="psA", bufs=3, space="PSUM") as psA,
                tc.tile_pool(name="psT", bufs=2, space="PSUM") as psT,
            ):
                wq_sb = ph1.tile([P, CT, G], MM_DT, tag="wqs")
                wk_sb = ph1.tile([P, CT, G], MM_DT, tag="wks")
                wv_sb = ph1.tile([P, CT, G], MM_DT, tag="wvs")
                wqv = wqT_d[:, :].rearrange("(ct p) g -> ct p g", p=P)
                wkv = wkT_d[:, :].rearrange("(ct p) g -> ct p g", p=P)
                wvv = wvT_d[:, :].rearrange("(ct p) g -> ct p g", p=P)
                for ct in range(CT):
                    nc.gpsimd.dma_start(out=wq_sb[:, ct, :], in_=wqv[ct])
                    nc.gpsimd.dma_start(out=wk_sb[:, ct, :], in_=wkv[ct])
                    nc.gpsimd.dma_start(out=wv_sb[:, ct, :], in_=wvv[ct])
                # emitted after the weight loads so the identity build and the
                # small loads don't block the gpsimd DMA queue at startup
                nc.gpsimd.dma_start(
                    out=mask_sb, in_=mask_d[:].rearrange("(j p) -> p j", p=P)
                )
                for nm, t in vec_sb.items():
                    nc.gpsimd.dma_start(
                        out=t, in_=_bcast_row(vec_d[VROW[nm] : VROW[nm] + 1, :], P)
                    )
                make_identity(nc, ident)
                xT_sb = ph1.tile([P, CT, S1], MM_DT, tag="xTs")
                xv = xT_d[:, :].rearrange("(ct p) i -> ct p i", p=P)
                yT_sb = ph1.tile([P, CT, S2P], MM_DT, tag="yTs")
                yv = yT_d[:, :].rearrange("(ct p) j -> ct p j", p=P)
                # first halves of every ct-chunk land before any second half,
                # so the first projection i-tiles start ~10us earlier
                dengs = (nc.sync, nc.scalar, nc.gpsimd)
                di = 0
                h1, h2_ = S1 // 2, S2P // 2
                for half in range(2):
                    xs = slice(0, h1) if half == 0 else slice(h1, S1)
                    ys = slice(0, h2_) if half == 0 else slice(h2_, S2P)
                    for ct in range(CT):
                        dengs[di % 3].dma_start(
                            out=xT_sb[:, ct, xs], in_=xv[ct][:, xs]
                        )
                        dengs[(di + 1) % 3].dma_start(
                            out=yT_sb[:, ct, ys], in_=yv[ct][:, ys]
                        )
                        di += 2

                qraw = ph1.tile([P, NI, G], F32, tag="qraw")
                smvq = ph1.tile([P, NI, 4, 2], F32, tag="smvq")
                rs_q, nm_q = ln_project(
                    xT_sb, wq_sb, qraw, smvq, NI, 1.0 / math.sqrt(D), "bq"
                )
                ln_apply_transpose(qraw, rs_q, nm_q, NI, "qw", "qb", qT)
                kraw = ph1.tile([P, NJ, G], F32, tag="kraw")
                smvk = ph1.tile([P, NJ, 4, 2], F32, tag="smvk")
                rs_k, nm_k = ln_project(yT_sb, wk_sb, kraw, smvk, NJ, 1.0, "bk")

                # v projection (no LN, no transpose; strided 65-col layout)
                for jt in range(NJ):
                    ps = psA.tile([P, G], F32, tag="psA")
                    for ct in range(CT):
                        nc.tensor.matmul(
                            ps,
                            lhsT=yT_sb[:, ct, jt * P : (jt + 1) * P],
                            rhs=wv_sb[:, ct, :],
                            start=(ct == 0),
                            stop=(ct == CT - 1),
                        )
                    ps3 = ps.rearrange("p (h e) -> p h e", e=D)
                    vdst = v4[:, jt, :, 0:D]
                    if "bv" in vec_sb:
                        bv3 = vec_sb["bv"].rearrange("p (h e) -> p h e", e=D)
                        nc.vector.tensor_add(out=vdst, in0=ps3, in1=bv3)
                    else:
                        nc.vector.tensor_copy(out=vdst, in_=ps3)

                ln_apply_transpose(kraw, rs_k, nm_k, NJ, "kw", "kb", kT)

            # -------- Phase 3+4: attention + per-block output projection -----
            with (
                tc.tile_pool(name="pp", bufs=2) as ppool,
                tc.tile_pool(name="attw", bufs=3) as attw,
                tc.tile_pool(name="attden", bufs=1) as attden,
                tc.tile_pool(name="ow", bufs=3) as ow,
                tc.tile_pool(name="dram", bufs=1, space="DRAM") as dramp,
                tc.tile_pool(name="psS", bufs=3, space="PSUM") as psS,
                tc.tile_pool(name="psC", bufs=1, space="PSUM") as psC,
                tc.tile_pool(name="psO", bufs=1, space="PSUM") as psO,
            ):
                NU = NIB * 2 * 2 * NC2
                rec_dram = dramp.tile([NU, 512], F32, tag="rec_dram")
                wo_sb = attden.tile([P, 2, C], MM_DT, tag="wo")
                wov = woT_d[:, :].rearrange("(k p) c -> k p c", p=P)
                for kt in range(2):
                    nc.gpsimd.dma_start(out=wo_sb[:, kt, :], in_=wov[kt])

                blocks = [(ib, hp) for ib in range(NIB) for hp in range(2)]
                pts_store = {}

                def emit_scores_jt(n, jt, pts):
                    ib, hp = blocks[n]
                    for h2 in range(2):
                        ps = psS.tile([P, IBW], F32, tag="ps_s", name="ps")
                        for cc in range(NC2):
                            c0 = ib * IBW + cc * 512
                            nc.tensor.matmul(
                                ps[:, cc * 512 : (cc + 1) * 512],
                                lhsT=kT[hp][
                                    h2 * D : (h2 + 1) * D, jt * P : (jt + 1) * P
                                ],
                                rhs=qT[hp][h2 * D : (h2 + 1) * D, c0 : c0 + 512],
                                start=True,
                                stop=True,
                            )
                        nc.scalar.activation(
                            out=pts[h2][:, jt, :],
                            in_=ps,
                            func=AF.Exp,
                            bias=mask_sb[:, jt : jt + 1],
                            scale=1.0,
                        )

                def emit_ctx_steps(n, state, nsteps):
                    """Advance block n's pv/normalize by ~nsteps matmuls."""
                    ib, hp = blocks[n]
                    pts = pts_store[n]
                    for _ in range(nsteps):
                        h2, cc, jt = state["pos"]
                        if h2 == 2:
                            return
                        hg = hp * 2 + h2
                        if jt == 0:
                            state["pc"] = psC.tile(
                                [D + 1, 512], F32, tag="ps_c", name="pc"
                            )
                        nc.tensor.matmul(
                            state["pc"],
                            lhsT=v_sb[:, jt, hg * (D + 1) : (hg + 1) * (D + 1)],
                            rhs=pts[h2][:, jt, cc * 512 : (cc + 1) * 512],
                            start=(jt == 0),
                            stop=(jt == NJ - 1),
                        )
                        if jt == NJ - 1:
                            uu = h2 * NC2 + cc
                            nc.vector.tensor_copy(
                                out=state["den"][32 * uu : 32 * uu + 1, :],
                                in_=state["pc"][D : D + 1, :],
                            )
                            cu = attw.tile([D, 512], F32, tag=f"cu{uu}", name="cu")
                            nc.vector.tensor_copy(out=cu, in_=state["pc"][0:D, :])
                            state["cus"][(h2, cc)] = cu
                            state["pos"] = (h2 + (cc + 1) // NC2, (cc + 1) % NC2, 0)
                        else:
                            state["pos"] = (h2, cc, jt + 1)

                def finish_ctx(n, state):
                    ib, hp = blocks[n]
                    while state["pos"][0] != 2:
                        emit_ctx_steps(n, state, 1000)
                    pts_store.pop(n)
                    u0 = (ib * 2 + hp) * 2 * NC2
                    rec_blk = attw.tile([P, 512], F32, tag="rec_blk")
                    nc.vector.reciprocal(out=rec_blk, in_=state["den"])
                    rec_rows = rec_blk.rearrange("(a b) f -> a b f", b=32)[:, 0, :]
                    nc.gpsimd.dma_start(out=rec_dram[u0 : u0 + 2 * NC2, :], in_=rec_rows)
                    for h2 in range(2):
                        for cc in range(NC2):
                            u = u0 + h2 * NC2 + cc
                            rec = attw.tile([D, 512], F32, tag="rec")
                            nc.gpsimd.dma_start(
                                out=rec, in_=_bcast_row(rec_dram[u : u + 1, :], D)
                            )
                            c0 = ib * IBW + cc * 512
                            nc.vector.tensor_mul(
                                out=ctxT[hp][h2 * D : (h2 + 1) * D, c0 : c0 + 512],
                                in0=state["cus"][(h2, cc)],
                                in1=rec,
                            )
                    if hp == 1:
                        # output projection for this i-block (both head pairs
                        # of ib done); overlaps the next block's scores
                        for it in range(ib * IBW // P, (ib + 1) * IBW // P):
                            ot = ow.tile([P, C], F32, tag="ot")
                            for oc in range(C // 512):
                                po = psO.tile([P, 512], F32, tag="ps_o")
                                for kt in range(2):
                                    nc.tensor.matmul(
                                        po,
                                        lhsT=ctxT[kt][:, it * P : (it + 1) * P],
                                        rhs=wo_sb[:, kt, oc * 512 : (oc + 1) * 512],
                                        start=(kt == 0),
                                        stop=(kt == 1),
                                    )
                                nc.vector.tensor_copy(
                                    out=ot[:, oc * 512 : (oc + 1) * 512], in_=po
                                )
                            deng = (nc.sync, nc.scalar, nc.gpsimd)[it % 3]
                            deng.dma_start(
                                out=out_d[it * P : (it + 1) * P, :], in_=ot
                            )

                def new_state():
                    den = attw.tile([P, 512], F32, tag="den_blk", name="den")
                    nc.vector.memset(den, 1.0)
                    return {"pos": (0, 0, 0), "cus": {}, "den": den, "pc": None}

                def new_pts(n):
                    pts_store[n] = [
                        ppool.tile([P, NJ, IBW], PV_DT, tag=f"p{h2}", name=f"p{h2}")
                        for h2 in range(2)
                    ]
                    return pts_store[n]

                # software pipeline: block n's scores interleave (at j-tile
                # granularity) with block n-1's pv accumulations, keeping the
                # PE stream dense enough that HAM stays at full clock
                nmm_per_step = (2 * 2 * NC2 * NJ + NJ - 1) // NJ
                pts_cur = new_pts(0)
                for jt in range(NJ):
                    emit_scores_jt(0, jt, pts_cur)
                for n in range(1, len(blocks) + 1):
                    state = new_state()
                    if n < len(blocks):
                        pts_next = new_pts(n)
                        for jt in range(NJ):
                            emit_scores_jt(n, jt, pts_next)
                            emit_ctx_steps(n - 1, state, nmm_per_step)
                    finish_ctx(n - 1, state)

    nc.finalize()
    return nc


def _ensure_axon_hooks():
    """bass_utils imports antenv.axon_hooks when BASS_TRACE is set; agent
    images lack that module, so provide a no-op stub (trace degrades)."""
    try:
        import antenv.axon_hooks  # noqa: F401
    except ImportError:
        import sys
        import types

        import antenv  # noqa: F401

        mod = types.ModuleType("antenv.axon_hooks")
        mod._hook = None
        mod.set_axon_ntff_profile_hook = lambda h: setattr(mod, "_hook", h)
        mod.get_axon_ntff_profile_hook = lambda: mod._hook
        sys.modules["antenv.axon_hooks"] = mod


def kernel(x, y, padding_mask, Wq, bq, Wkv, bkv, qn_w, qn_b, kn_w, kn_b, Wo, bo):
    global LAST_EXEC_NS
    _ensure_axon_hooks()
    x = np.asarray(x, dtype=np.float32)
    y = np.asarray(y, dtype=np.float32)
    padding_mask = np.asarray(padding_mask)
    Wq = np.asarray(Wq, dtype=np.float32)
    bq = np.asarray(bq, dtype=np.float32)
    Wkv = np.asarray(Wkv, dtype=np.float32)
    bkv = np.asarray(bkv, dtype=np.float32)
    qn_w = np.asarray(qn_w, dtype=np.float32)
    qn_b = np.asarray(qn_b, dtype=np.float32)
    kn_w = np.asarray(kn_w, dtype=np.float32)
    kn_b = np.asarray(kn_b, dtype=np.float32)
    Wo = np.asarray(Wo, dtype=np.float32)
    bo = np.asarray(bo, dtype=np.float32)

    b, S1, C = x.shape
    assert b == 2 and C % 16 == 0
    d = C // 16
    scale = d**-0.5
    G = 4 * d  # 4 heads per core

    idxs = [np.flatnonzero(padding_mask[bi]) for bi in range(b)]
    s2v = [len(ix) for ix in idxs]
    S2P = max(P, ((max(s2v) + P - 1) // P) * P)

    flags = {
        "bq": bool(np.any(bq)),
        "bk": bool(np.any(bkv[:C])),
        "bv": bool(np.any(bkv[C:])),
        "qw": not bool(np.all(qn_w == 1.0)),
        "qb": bool(np.any(qn_b)),
        "kw": not bool(np.all(kn_w == 1.0)),
        "kb": bool(np.any(kn_b)),
    }

    nc = _build_nc(S1, S2P, C, flags)

    mm_np = {BF16: ml_dtypes.bfloat16, F32R: np.float32, F32: np.float32}[MM_DT]
    in_maps = []
    yTs = []
    for bi in range(b):
        yv = np.zeros((S2P, C), np.float32)
        yv[: s2v[bi]] = y[bi][idxs[bi]]
        yTs.append(np.ascontiguousarray(yv.T).astype(mm_np))
    xTs = [np.ascontiguousarray(x[bi].T).astype(mm_np) for bi in range(b)]
    for core in range(8):
        bc, g = divmod(core, 4)
        rows = slice(g * G, (g + 1) * G)
        vecs = np.zeros((8, G), np.float32)
        vecs[0] = bq[rows]
        vecs[1] = bkv[rows]
        vecs[2] = bkv[C + g * G : C + (g + 1) * G]
        vecs[3] = np.tile(qn_w, 4)
        vecs[4] = np.tile(qn_b * scale, 4)
        vecs[5] = np.tile(kn_w, 4)
        vecs[6] = np.tile(kn_b, 4)
        mb = np.zeros((S2P,), np.float32)
        mb[s2v[bc] :] = MASK_NEG
        in_maps.append(
            {
                "xT": xTs[bc],
                "yT": yTs[bc],
                "wqT": np.ascontiguousarray(Wq[rows, :].T).astype(mm_np),
                "wkT": np.ascontiguousarray(Wkv[rows, :].T).astype(mm_np),
                "wvT": np.ascontiguousarray(
                    Wkv[C + g * G : C + (g + 1) * G, :].T
                ).astype(mm_np),
                "woT": np.ascontiguousarray(Wo[:, rows].T).astype(mm_np),
                "vec": vecs,
                "maskb": mb,
            }
        )

    res = run_bass_kernel_spmd(nc, in_maps, core_ids=list(range(8)))
    LAST_EXEC_NS = res.exec_time_ns

    out = np.zeros((b, S1, C), np.float32)
    for core in range(8):
        out[core // 4] += res.results[core]["out"]
    out += bo
    return out



# revision 22
# speedup vs baseline: 1.0607x; 1.0607x over previous
"""Trainium2 Bass kernel for nn_CrossAttention (b=2, s1=2048, s2=3072, 16 heads, d=64).

Sharding: 8 cores = 2 batches x 4 head-groups (4 heads each).

Structure (v2):
  - kv projection fused into one N=512 matmul stream; LN stats via bn_stats.
  - k is never normalized on-chip: normalized q is zero-mean per head, so
    q_n.(k - mu_k) == q_n.k, and the per-key rsqrt(var) folds into the exp
    (per-partition scale operand).  k LayerNorm therefore costs only stats.
  - padding handled by compacting valid keys on host (zero pad): pad scores
    are exactly 0, the v "ones column" is 0 on pad rows, so softmax
    numerator/denominator are unaffected - no mask bias anywhere.
  - exp is split: ~2/3 of (jt, h2) units on ACT (exact exp, per-key scale),
    ~1/3 on DVE via a Schraudolph int16 bit-trick writing bf16 probabilities
    directly (one tensor_scalar per tile).
  - softmax denominators come from the v ones-column; reciprocal via
    reciprocal_approx_fast; broadcast via a DRAM round-trip; one
    tensor_tensor multiply normalizes ctx straight out of PSUM.
  - per-i-block output projection overlapped with the next block's scores.
Host sums the 4 partial outputs per batch and adds bo.
"""

import math
import os

import ml_dtypes
import numpy as np

import concourse.bacc as bacc
import concourse.bass as bass
import concourse.tile as tile
from concourse import mybir
from concourse.bass_utils import run_bass_kernel_spmd
from concourse.masks import make_identity

F32 = mybir.dt.float32
BF16 = mybir.dt.bfloat16
I16 = mybir.dt.int16

P = 128
D = 64
EPS = 1e-6
LOG2E = 1.4426950408889634
# exp bit-trick: i16 = round(s*rs*128*log2e + 127*128), bitcast bf16.
EXPC = 127.0 * 128.0

# every APPROX_MOD-th exp unit goes to the DVE bit-trick path
APPROX_MOD = int(os.environ.get("K_APPROX_MOD", "3"))

LAST_EXEC_NS = None


def _bcast_row(ap, nparts):
    return bass.AP(
        tensor=ap.tensor, offset=ap.offset, ap=[[0, nparts]] + list(ap.ap[1:])
    )


def _build_nc(S1, S2P, C, flags, kwc=1.0):
    G = 4 * D  # 256 channels per core (4 heads)
    NI = S1 // P
    NJ = S2P // P
    CT = C // P
    IBW = 1024
    NIB = S1 // IBW
    NC2 = IBW // 512
    AF = mybir.ActivationFunctionType
    OP = mybir.AluOpType

    # k-side handling: 0 = stats only (scale folded into exp)
    #                  1 = center-only apply (kn_w non-const)
    #                  2 = full LN apply (qn_b != 0 breaks the zero-mean trick)
    if flags["qb"]:
        KMODE = 2
    elif flags["kw"]:
        KMODE = 1
    else:
        KMODE = 0

    nc = bacc.Bacc("TRN2", target_bir_lowering=False, debug=False)

    xT_d = nc.dram_tensor("xT", [C, S1], BF16, kind="ExternalInput")
    yT_d = nc.dram_tensor("yT", [C, S2P], BF16, kind="ExternalInput")
    wqT_d = nc.dram_tensor("wqT", [C, G], BF16, kind="ExternalInput")
    wkvT_d = nc.dram_tensor("wkvT", [C, 2 * G], BF16, kind="ExternalInput")
    woT_d = nc.dram_tensor("woT", [G, C], BF16, kind="ExternalInput")
    vec_d = nc.dram_tensor("vec", [8, G], F32, kind="ExternalInput")
    vones_d = nc.dram_tensor("vones", [S2P], F32, kind="ExternalInput")
    out_d = nc.dram_tensor("out", [S1, C], F32, kind="ExternalOutput")
    DBG = bool(os.environ.get("K_DEBUG"))
    if DBG:
        dbg_d = {
            "d_qT0": nc.dram_tensor("d_qT0", [P, S1], BF16, kind="ExternalOutput"),
            "d_kT0": nc.dram_tensor("d_kT0", [P, S2P], BF16, kind="ExternalOutput"),
            "d_v": nc.dram_tensor(
                "d_v", [P, NJ, 4 * (D + 1)], BF16, kind="ExternalOutput"
            ),
            "d_rske": nc.dram_tensor("d_rske", [P, NJ, 4], F32, kind="ExternalOutput"),
            "d_ctxT0": nc.dram_tensor("d_ctxT0", [P, S1], BF16, kind="ExternalOutput"),
            "d_pts": nc.dram_tensor("d_pts", [2, P, NJ, 1024], BF16, kind="ExternalOutput"),
            "d_den": nc.dram_tensor("d_den", [16, 512], F32, kind="ExternalOutput"),
            "d_rec": nc.dram_tensor("d_rec", [16, 512], F32, kind="ExternalOutput"),
        }

    VROW = {"bq": 0, "bk": 1, "bv": 2, "qw": 3, "qb": 4, "kw": 5, "kb": 6}

    with tile.TileContext(nc) as tc:
        with (
            tc.tile_pool(name="singles", bufs=1) as singles,
            tc.tile_pool(name="persist", bufs=1) as persist,
        ):
            ident = singles.tile([P, P], BF16, tag="ident")
            eps_sb = singles.tile([P, 1], F32, tag="eps")
            nc.vector.memset(eps_sb, 64.0 * EPS)
            dummy = singles.tile([P, 1], F32, tag="dummy")
            vones_sb = singles.tile([P, NJ], F32, tag="vones")
            vec_sb = {}
            for nm in [k for k, use in flags.items() if use]:
                vec_sb[nm] = singles.tile([P, G], F32, tag=f"vec_{nm}", name=f"v{nm}")

            kT = [
                persist.tile([P, S2P], BF16, tag=f"kT{i}", name=f"kT{i}")
                for i in range(2)
            ]
            qT = [
                persist.tile([P, S1], BF16, tag=f"qT{i}", name=f"qT{i}")
                for i in range(2)
            ]
            v_sb = persist.tile([P, NJ, 4 * (D + 1)], BF16, tag="v")
            v4 = v_sb.rearrange("p j (h e) -> p j h e", e=D + 1)
            ctxT = [
                persist.tile([P, S1], BF16, tag=f"ctxT{i}", name=f"ctxT{i}")
                for i in range(2)
            ]
            wo_sb = persist.tile([P, 2, C], BF16, tag="wo")
            # per-key folded LN scales for the exp paths, [P, NJ, 4]
            rsk_exp = persist.tile([P, NJ, 4], F32, tag="rske")
            rsk_apx = persist.tile([P, NJ, 4], F32, tag="rska")

            # ---------------- Phase 1: projections + LN stats ----------------
            with (
                tc.tile_pool(name="ph1", bufs=1) as ph1,
                tc.tile_pool(name="work", bufs=4) as work,
                tc.tile_pool(name="qaw", bufs=3) as qaw,
                tc.tile_pool(name="psA", bufs=3, space="PSUM") as psA,
                tc.tile_pool(name="psT", bufs=2, space="PSUM") as psT,
            ):
                wkv_sb = ph1.tile([P, CT, 2 * G], BF16, tag="wkvs")
                wq_sb = ph1.tile([P, CT, G], BF16, tag="wqs")
                wkvv = wkvT_d[:, :].rearrange("(ct p) g -> ct p g", p=P)
                wqv = wqT_d[:, :].rearrange("(ct p) g -> ct p g", p=P)
                for ct in range(CT):
                    nc.gpsimd.dma_start(out=wkv_sb[:, ct, :], in_=wkvv[ct])
                nc.gpsimd.dma_start(
                    out=vones_sb, in_=vones_d[:].rearrange("(j p) -> p j", p=P)
                )
                for nm, t in vec_sb.items():
                    nc.gpsimd.dma_start(
                        out=t, in_=_bcast_row(vec_d[VROW[nm] : VROW[nm] + 1, :], P)
                    )
                make_identity(nc, ident)
                yT_sb = ph1.tile([P, CT, S2P], BF16, tag="yTs")
                yv = yT_d[:, :].rearrange("(ct p) j -> ct p j", p=P)
                xT_sb = ph1.tile([P, CT, S1], BF16, tag="xTs")
                xv = xT_d[:, :].rearrange("(ct p) i -> ct p i", p=P)
                dengs = (nc.sync, nc.scalar, nc.gpsimd)
                di = 0
                h1, h2_ = S1 // 2, S2P // 2
                # y first (kv proj runs first), then wq, then x halves
                for half in range(2):
                    ys = slice(0, h2_) if half == 0 else slice(h2_, S2P)
                    for ct in range(CT):
                        dengs[di % 3].dma_start(out=yT_sb[:, ct, ys], in_=yv[ct][:, ys])
                        di += 1
                for ct in range(CT):
                    nc.gpsimd.dma_start(out=wq_sb[:, ct, :], in_=wqv[ct])
                for half in range(2):
                    xs = slice(0, h1) if half == 0 else slice(h1, S1)
                    for ct in range(CT):
                        dengs[di % 3].dma_start(out=xT_sb[:, ct, xs], in_=xv[ct][:, xs])
                        di += 1
                wov = woT_d[:, :].rearrange("(k p) c -> k p c", p=P)
                for kt in range(2):
                    nc.gpsimd.dma_start(out=wo_sb[:, kt, :], in_=wov[kt])

                kraw = ph1.tile([P, NJ, G], BF16, tag="kraw")
                kbn = ph1.tile([P, NJ, 4, 6], F32, tag="kbn")
                qraw = ph1.tile([P, NI, G], F32, tag="qraw")
                qbn = ph1.tile([P, NI, 4, 6], F32, tag="qbn")

                # ---- kv projection + k stats + v staging
                for jt in range(NJ):
                    ps = psA.tile([P, 2 * G], F32, tag="psA")
                    for ct in range(CT):
                        nc.tensor.matmul(
                            ps,
                            lhsT=yT_sb[:, ct, jt * P : (jt + 1) * P],
                            rhs=wkv_sb[:, ct, :],
                            start=(ct == 0),
                            stop=(ct == CT - 1),
                        )
                    kps = ps[:, 0:G]
                    if "bk" in vec_sb:
                        nc.scalar.activation(
                            out=kraw[:, jt, :], in_=kps, func=AF.Copy, bias=0.0
                        )
                        # bias folded later is impossible for bf16 raw; add now
                        nc.vector.tensor_add(
                            out=kraw[:, jt, :], in0=kraw[:, jt, :], in1=vec_sb["bk"]
                        )
                    else:
                        nc.scalar.copy(out=kraw[:, jt, :], in_=kps)
                    for h in range(4):
                        nc.vector.bn_stats(
                            out=kbn[:, jt, h, :], in_=kps[:, h * D : (h + 1) * D]
                        )
                    vps = ps[:, G : 2 * G].rearrange("p (h e) -> p h e", e=D)
                    if "bv" in vec_sb:
                        bv3 = vec_sb["bv"].rearrange("p (h e) -> p h e", e=D)
                        nc.vector.tensor_add(out=v4[:, jt, :, 0:D], in0=vps, in1=bv3)
                    else:
                        nc.vector.tensor_copy(out=v4[:, jt, :, 0:D], in_=vps)
                # v ones-column (0 on pad rows -> exact denominators)
                for h in range(4):
                    nc.vector.tensor_copy(out=v4[:, :, h, D], in_=vones_sb)

                # ---- k LN batch: rs_k vectors (and mu if centering)
                def ln_batch(bn, n4, tagp):
                    """bn: [P, nt, 4, 6] -> (rs_raw [P, n4], msum [P, n4]).
                    rs_raw = 1/(8*sqrt(var+eps)); mu = 0.5*msum."""
                    bnf = bn.rearrange("p t h s -> p (t h) s")
                    msum = work.tile([P, n4], F32, tag=f"{tagp}ms")
                    nc.vector.tensor_add(
                        out=msum, in0=bnf[:, :, 1], in1=bnf[:, :, 4]
                    )
                    mdif = work.tile([P, n4], F32, tag=f"{tagp}md")
                    nc.vector.tensor_sub(
                        out=mdif, in0=bnf[:, :, 1], in1=bnf[:, :, 4]
                    )
                    cvs = work.tile([P, n4], F32, tag=f"{tagp}cv")
                    nc.vector.tensor_add(out=cvs, in0=bnf[:, :, 2], in1=bnf[:, :, 5])
                    m2 = work.tile([P, n4], F32, tag=f"{tagp}m2")
                    nc.vector.tensor_mul(out=m2, in0=mdif, in1=mdif)
                    var64 = work.tile([P, n4], F32, tag=f"{tagp}va")
                    nc.vector.scalar_tensor_tensor(
                        out=var64, in0=m2, scalar=16.0, in1=cvs, op0=OP.mult,
                        op1=OP.add,
                    )
                    sd = work.tile([P, n4], F32, tag=f"{tagp}sd")
                    nc.scalar.activation(
                        out=sd, in_=var64, func=AF.Sqrt, bias=eps_sb, scale=1.0
                    )
                    rs = work.tile([P, n4], F32, tag=f"{tagp}rs")
                    nc.vector.reciprocal_approx_fast(out=rs, in_=sd)
                    return rs, msum

                krs, kms = ln_batch(kbn, NJ * 4, "k")
                # folded exp scales: rs_true = 8*rs_raw
                rske_f = rsk_exp.rearrange("p t h -> p (t h)")
                rska_f = rsk_apx.rearrange("p t h -> p (t h)")
                if KMODE == 2:
                    nc.vector.memset(rske_f, 1.0)
                    nc.vector.memset(rska_f, 128.0 * LOG2E)
                else:
                    nc.vector.tensor_scalar_mul(out=rske_f, in0=krs, scalar1=8.0 * kwc)
                    nc.vector.tensor_scalar_mul(
                        out=rska_f, in0=krs, scalar1=1024.0 * LOG2E * kwc
                    )

                # ---- k transposes (optionally centered / fully applied)
                if KMODE == 0:
                    ksrc = kraw
                else:
                    knm = work.tile([P, NJ * 4], F32, tag="knm")
                    if KMODE == 1:
                        # -mu
                        nc.vector.tensor_scalar_mul(out=knm, in0=kms, scalar1=-0.5)
                        krss = None
                    else:
                        # full: rs8 = 8*rs_raw ; nm = -0.5*msum*rs8
                        krss = work.tile([P, NJ * 4], F32, tag="krss")
                        nc.vector.tensor_scalar_mul(out=krss, in0=krs, scalar1=8.0)
                        nc.vector.tensor_mul(out=knm, in0=kms, in1=krss)
                        nc.vector.tensor_scalar_mul(out=knm, in0=knm, scalar1=-0.5)
                    ksrc = ph1.tile([P, NJ, G], BF16, tag="kapp")
                    for jt in range(NJ):
                        for h in range(4):
                            i4 = jt * 4 + h
                            sl = slice(h * D, (h + 1) * D)
                            eng = nc.gpsimd if jt % 2 == 0 else nc.vector
                            eng.tensor_scalar(
                                out=ksrc[:, jt, sl],
                                in0=kraw[:, jt, sl],
                                scalar1=(1.0 if KMODE == 1 else krss[:, i4 : i4 + 1]),
                                scalar2=knm[:, i4 : i4 + 1],
                                op0=OP.mult,
                                op1=OP.add,
                            )
                        if KMODE == 2 and "kw" in vec_sb:
                            nc.vector.tensor_mul(
                                out=ksrc[:, jt, :], in0=ksrc[:, jt, :],
                                in1=vec_sb["kw"],
                            )
                        if KMODE == 2 and "kb" in vec_sb:
                            nc.vector.tensor_add(
                                out=ksrc[:, jt, :], in0=ksrc[:, jt, :],
                                in1=vec_sb["kb"],
                            )
                for jt in range(NJ):
                    for hp in range(2):
                        pt = psT.tile([P, P], BF16, tag="ptr")
                        nc.tensor.transpose(
                            pt, ksrc[:, jt, hp * P : (hp + 1) * P], ident
                        )
                        nc.scalar.copy(
                            out=kT[hp][:, jt * P : (jt + 1) * P], in_=pt
                        )

                # ---- q projection + stats
                for it in range(NI):
                    ps = psA.tile([P, 2 * G], F32, tag="psA")
                    for ct in range(CT):
                        nc.tensor.matmul(
                            ps[:, 0:G],
                            lhsT=xT_sb[:, ct, it * P : (it + 1) * P],
                            rhs=wq_sb[:, ct, :],
                            start=(ct == 0),
                            stop=(ct == CT - 1),
                        )
                    qps = ps[:, 0:G]
                    if "bq" in vec_sb:
                        nc.vector.tensor_add(
                            out=qraw[:, it, :], in0=qps, in1=vec_sb["bq"]
                        )
                    else:
                        nc.scalar.copy(out=qraw[:, it, :], in_=qps)
                    for h in range(4):
                        src = qraw[:, it, :] if "bq" in vec_sb else qps
                        nc.vector.bn_stats(
                            out=qbn[:, it, h, :], in_=src[:, h * D : (h + 1) * D]
                        )

                qrs, qms = ln_batch(qbn, NI * 4, "q")
                # rs_q*scale = rs_raw exactly (scale = 1/8 = 1/sqrt(D) * ... )
                qnm = work.tile([P, NI * 4], F32, tag="qnm")
                nc.vector.tensor_mul(out=qnm, in0=qms, in1=qrs)
                nc.vector.tensor_scalar_mul(out=qnm, in0=qnm, scalar1=-0.5)
                # preload the exp table set while transposes run
                nc.scalar.activation(out=dummy, in_=eps_sb, func=AF.Exp, bias=0.0)

                # ---- q apply + transposes
                for it in range(NI):
                    qa = qaw.tile([P, G], BF16, tag="qa")
                    for h in range(4):
                        i4 = it * 4 + h
                        sl = slice(h * D, (h + 1) * D)
                        eng = nc.gpsimd if it % 2 == 0 else nc.vector
                        eng.tensor_scalar(
                            out=qa[:, sl],
                            in0=qraw[:, it, sl],
                            scalar1=qrs[:, i4 : i4 + 1],
                            scalar2=qnm[:, i4 : i4 + 1],
                            op0=OP.mult,
                            op1=OP.add,
                        )
                    if "qw" in vec_sb:
                        nc.vector.tensor_mul(out=qa, in0=qa, in1=vec_sb["qw"])
                    if "qb" in vec_sb:
                        nc.vector.tensor_add(out=qa, in0=qa, in1=vec_sb["qb"])
                    for hp in range(2):
                        pt = psT.tile([P, P], BF16, tag="ptr")
                        nc.tensor.transpose(pt, qa[:, hp * P : (hp + 1) * P], ident)
                        nc.scalar.copy(
                            out=qT[hp][:, it * P : (it + 1) * P], in_=pt
                        )

            # ---------------- Phase 2: attention + output projection ---------
            with (
                tc.tile_pool(name="pp", bufs=2) as ppool,
                tc.tile_pool(name="attw", bufs=3) as attw,
                tc.tile_pool(name="ow", bufs=3) as ow,
                tc.tile_pool(name="dram", bufs=1, space="DRAM") as dramp,
                tc.tile_pool(name="psS", bufs=2, space="PSUM") as psS,
                tc.tile_pool(name="psC", bufs=2, space="PSUM") as psC,
                tc.tile_pool(name="psO", bufs=2, space="PSUM") as psO,
            ):
                NU = NIB * 2 * 2 * NC2
                rec_dram = dramp.tile([NU, 512], F32, tag="rec_dram")


                blocks = [(ib, hp) for ib in range(NIB) for hp in range(2)]
                pts_store = {}

                def emit_scores_jt(n, jt, pts):
                    ib, hp = blocks[n]
                    for h2 in range(2):
                        hg = hp * 2 + h2
                        ps = psS.tile([P, IBW], F32, tag="ps_s", name="ps")
                        for cc in range(NC2):
                            c0 = ib * IBW + cc * 512
                            nc.tensor.matmul(
                                ps[:, cc * 512 : (cc + 1) * 512],
                                lhsT=kT[hp][
                                    h2 * D : (h2 + 1) * D, jt * P : (jt + 1) * P
                                ],
                                rhs=qT[hp][h2 * D : (h2 + 1) * D, c0 : c0 + 512],
                                start=True,
                                stop=True,
                            )
                        if (jt * 2 + h2) % APPROX_MOD == 1:
                            nc.vector.tensor_scalar(
                                out=pts[h2][:, jt, :].bitcast(I16),
                                in0=ps,
                                scalar1=rsk_apx[:, jt, hg : hg + 1],
                                scalar2=EXPC,
                                op0=OP.mult,
                                op1=OP.add,
                            )
                        else:
                            nc.scalar.activation(
                                out=pts[h2][:, jt, :],
                                in_=ps,
                                func=AF.Exp,
                                bias=0.0,
                                scale=rsk_exp[:, jt, hg : hg + 1],
                            )

                def finish_unit(n, h2, cc, pc):
                    """den -> rec -> DRAM bcast -> normalize ctx (from PSUM)."""
                    ib, hp = blocks[n]
                    u = ((ib * 2 + hp) * 2 + h2) * NC2 + cc
                    if DBG and u < 16:
                        dend = attw.tile([1, 512], F32, tag="dend", name="dend")
                        nc.vector.tensor_copy(out=dend, in_=pc[D : D + 1, :])
                        nc.sync.dma_start(out=dbg_d["d_den"][u : u + 1, :], in_=dend)
                    den_row = attw.tile([1, 512], F32, tag="den_row")
                    nc.vector.tensor_copy(out=den_row, in_=pc[D : D + 1, :])
                    rec_row = attw.tile([1, 512], F32, tag="rec_row")
                    nc.vector.reciprocal_approx_fast(out=rec_row, in_=den_row)
                    if DBG and u < 16:
                        nc.sync.dma_start(out=dbg_d["d_rec"][u : u + 1, :], in_=rec_row)
                    nc.gpsimd.dma_start(out=rec_dram[u : u + 1, :], in_=rec_row)
                    rec_bc = attw.tile([D, 512], F32, tag="rec_bc")
                    nc.gpsimd.dma_start(
                        out=rec_bc, in_=_bcast_row(rec_dram[u : u + 1, :], D)
                    )
                    c0 = ib * IBW + cc * 512
                    nc.vector.tensor_tensor(
                        out=ctxT[hp][h2 * D : (h2 + 1) * D, c0 : c0 + 512],
                        in0=pc[0:D, :],
                        in1=rec_bc,
                        op=OP.mult,
                    )

                def emit_ctx_steps(n, state, nsteps):
                    ib, hp = blocks[n]
                    pts = pts_store[n]
                    for _ in range(nsteps):
                        h2, cc, jt = state["pos"]
                        if h2 == 2:
                            return
                        hg = hp * 2 + h2
                        if jt == 0:
                            state["pc"] = psC.tile([D + 1, 512], F32, tag="ps_c", name="pc")
                        nc.tensor.matmul(
                            state["pc"],
                            lhsT=v_sb[:, jt, hg * (D + 1) : (hg + 1) * (D + 1)],
                            rhs=pts[h2][:, jt, cc * 512 : (cc + 1) * 512],
                            start=(jt == 0),
                            stop=(jt == NJ - 1),
                        )
                        if jt == NJ - 1:
                            finish_unit(n, h2, cc, state["pc"])
                            state["pos"] = (h2 + (cc + 1) // NC2, (cc + 1) % NC2, 0)
                        else:
                            state["pos"] = (h2, cc, jt + 1)

                def finish_block(n, state):
                    ib, hp = blocks[n]
                    while state["pos"][0] != 2:
                        emit_ctx_steps(n, state, 1000)
                    if not (DBG and n == len(blocks) - 1):
                        pts_store.pop(n)
                    if hp == 1:
                        for it in range(ib * IBW // P, (ib + 1) * IBW // P):
                            ot = ow.tile([P, C], F32, tag="ot")
                            for oc in range(C // 512):
                                po = psO.tile([P, 512], F32, tag="ps_o")
                                for kt in range(2):
                                    nc.tensor.matmul(
                                        po,
                                        lhsT=ctxT[kt][:, it * P : (it + 1) * P],
                                        rhs=wo_sb[:, kt, oc * 512 : (oc + 1) * 512],
                                        start=(kt == 0),
                                        stop=(kt == 1),
                                    )
                                nc.vector.tensor_copy(
                                    out=ot[:, oc * 512 : (oc + 1) * 512], in_=po
                                )
                            deng = (nc.sync, nc.scalar, nc.gpsimd)[it % 3]
                            deng.dma_start(out=out_d[it * P : (it + 1) * P, :], in_=ot)

                def new_pts(n):
                    pts_store[n] = [
                        ppool.tile([P, NJ, IBW], BF16, tag=f"p{h2}", name=f"p{h2}")
                        for h2 in range(2)
                    ]
                    return pts_store[n]

                nmm_per_step = (2 * 2 * NC2 * NJ + NJ - 1) // NJ
                pts_cur = new_pts(0)
                for jt in range(NJ):
                    emit_scores_jt(0, jt, pts_cur)
                for n in range(1, len(blocks) + 1):
                    state = {"pos": (0, 0, 0), "pc": None}
                    if n < len(blocks):
                        pts_next = new_pts(n)
                        for jt in range(NJ):
                            emit_scores_jt(n, jt, pts_next)
                            emit_ctx_steps(n - 1, state, nmm_per_step)
                    finish_block(n - 1, state)

                if DBG:
                    nc.sync.dma_start(out=dbg_d["d_qT0"][:, :], in_=qT[0])
                    nc.sync.dma_start(out=dbg_d["d_kT0"][:, :], in_=kT[0])
                    nc.sync.dma_start(out=dbg_d["d_v"][:, :, :], in_=v_sb)
                    nc.sync.dma_start(out=dbg_d["d_rske"][:, :, :], in_=rsk_exp)
                    nc.sync.dma_start(out=dbg_d["d_ctxT0"][:, :], in_=ctxT[0])
                    for h2 in range(2):
                        nc.sync.dma_start(
                            out=dbg_d["d_pts"][h2, :, 0:NJ, :],
                            in_=pts_store[len(blocks) - 1][h2],
                        )

    nc.finalize()
    return nc


def _ensure_axon_hooks():
    try:
        import antenv.axon_hooks  # noqa: F401
    except ImportError:
        import sys
        import types

        import antenv  # noqa: F401

        mod = types.ModuleType("antenv.axon_hooks")
        mod._hook = None
        mod.set_axon_ntff_profile_hook = lambda h: setattr(mod, "_hook", h)
        mod.get_axon_ntff_profile_hook = lambda: mod._hook
        sys.modules["antenv.axon_hooks"] = mod


def kernel(x, y, padding_mask, Wq, bq, Wkv, bkv, qn_w, qn_b, kn_w, kn_b, Wo, bo):
    global LAST_EXEC_NS
    _ensure_axon_hooks()
    x = np.asarray(x, dtype=np.float32)
    y = np.asarray(y, dtype=np.float32)
    padding_mask = np.asarray(padding_mask)
    Wq = np.asarray(Wq, dtype=np.float32)
    bq = np.asarray(bq, dtype=np.float32)
    Wkv = np.asarray(Wkv, dtype=np.float32)
    bkv = np.asarray(bkv, dtype=np.float32)
    qn_w = np.asarray(qn_w, dtype=np.float32)
    qn_b = np.asarray(qn_b, dtype=np.float32)
    kn_w = np.asarray(kn_w, dtype=np.float32)
    kn_b = np.asarray(kn_b, dtype=np.float32)
    Wo = np.asarray(Wo, dtype=np.float32)
    bo = np.asarray(bo, dtype=np.float32)

    b, S1, C = x.shape
    assert b == 2 and C % 16 == 0
    d = C // 16
    G = 4 * d

    idxs = [np.flatnonzero(padding_mask[bi]) for bi in range(b)]
    s2v = [len(ix) for ix in idxs]
    S2P = max(P, ((max(s2v) + P - 1) // P) * P)

    flags = {
        "bq": bool(np.any(bq)),
        "bk": bool(np.any(bkv[:C])),
        "bv": bool(np.any(bkv[C:])),
        "qw": not bool(np.all(qn_w == 1.0)),
        "qb": bool(np.any(qn_b)),
        "kw": not bool(np.all(kn_w == kn_w[0])),
        "kb": bool(np.any(kn_b)),
    }
    # kn_b is softmax-invariant unless qn_b is nonzero; constant kn_w folds
    # into the exp scale on host via kw_const.
    kw_const = float(kn_w[0]) if not flags["kw"] else 1.0

    nc = _build_nc(S1, S2P, C, flags, kwc=kw_const)

    bf = ml_dtypes.bfloat16
    in_maps = []
    yTs = []
    for bi in range(b):
        yv = np.zeros((S2P, C), np.float32)
        yv[: s2v[bi]] = y[bi][idxs[bi]]
        yTs.append(np.ascontiguousarray(yv.T).astype(bf))
    xTs = [np.ascontiguousarray(x[bi].T).astype(bf) for bi in range(b)]
    for core in range(8):
        bc, g = divmod(core, 4)
        rows = slice(g * G, (g + 1) * G)
        vecs = np.zeros((8, G), np.float32)
        vecs[0] = bq[rows]
        vecs[1] = bkv[rows]
        vecs[2] = bkv[C + g * G : C + (g + 1) * G]
        vecs[3] = np.tile(qn_w * (kn_w if not flags["qb"] and flags["kw"] else 1.0), 4)
        vecs[4] = np.tile(qn_b * (d ** -0.5), 4)
        vecs[5] = np.tile(kn_w, 4)
        vecs[6] = np.tile(kn_b, 4)
        vones = np.zeros((S2P,), np.float32)
        vones[: s2v[bc]] = 1.0
        wkvT = np.concatenate(
            [Wkv[rows, :].T, Wkv[C + g * G : C + (g + 1) * G, :].T], axis=1
        )
        in_maps.append(
            {
                "xT": xTs[bc],
                "yT": yTs[bc],
                "wqT": np.ascontiguousarray(Wq[rows, :].T).astype(bf),
                "wkvT": np.ascontiguousarray(wkvT).astype(bf),
                "woT": np.ascontiguousarray(Wo[:, rows].T).astype(bf),
                "vec": vecs,
                "vones": vones,
            }
        )

    res = run_bass_kernel_spmd(nc, in_maps, core_ids=list(range(8)))
    LAST_EXEC_NS = res.exec_time_ns

    out = np.zeros((b, S1, C), np.float32)
    for core in range(8):
        out[core // 4] += res.results[core]["out"]
    out += bo
    return out
